# revision 11
# baseline (speedup 1.0000x reference)
"""BioLatentMoE layer on 8 TRN2 NeuronCores.

Strategy: token-parallel (1024 tokens/core), dense experts with per-expert
combine weights (comb==0 for non-selected tokens), all matmuls in float32r
(4x fp32 PE rate), feature-major activation layout [feature, token] so the
whole chain runs without on-device transposes of activations.

Host side: shards h by token, folds norm_w into router/latent_down/shared_w1/
shared_gate weights (exact reassociation), folds vth into w2 weights
(spike(g,vth)*lin @ w2 == ((g>=vth)*lin) @ (vth*w2)), transposes h shards to
feature-major, and assembles the output + aux loss from per-core partials.
"""
import sys

sys.path.insert(0, '/opt/trn_rl_repo')

import numpy as np
import concourse.bass as bass
import concourse.bacc as bacc
import concourse.mybir as mybir
import concourse.tile as tile
from concourse.bass_utils import run_bass_kernel_spmd
from concourse import masks

S, B, D = 2048, 4, 2048
L, E, TOPK = 256, 32, 4
EH, SH = 1024, 2048
N = S * B
NCORES = 8
T = N // NCORES          # 1024 tokens per core
DC = D // 128            # 16
LC = L // 128            # 2
EHC = EH // 128          # 8
SHC = SH // 128          # 16
NH = 2                   # token halves of 512 (psum bank = 512 fp32)
HALF = T // NH           # 512

f32 = mybir.dt.float32
f32r = mybir.dt.float32r
AX = mybir.AxisListType
OP = mybir.AluOpType
AF = mybir.ActivationFunctionType


def build_nc():
    nc = bacc.Bacc("TRN2", target_bir_lowering=False, debug=False,
                   num_devices=NCORES)

    def din(name, shape, dt=f32r):
        return nc.dram_tensor(name, shape, dt, kind="ExternalInput").ap()

    hT_d = din("hT", [D, T], f32)
    rwf_d = din("rwf", [D, E])
    rb_d = din("rb", [E, 1], f32)
    ldf_d = din("ldf", [D, L])
    lu_d = din("lu", [L, D])
    ew1_d = din("ew1", [E, L, 2 * EH])
    evth_d = din("evth", [E, EH], f32)
    ew2s_d = din("ew2s", [E, EH, L])
    sw1f_d = din("sw1f", [D, 2 * SH])
    svth_d = din("svth", [SH], f32)
    sw2s_d = din("sw2s", [SH, D])
    sgwf_d = din("sgwf", [D, 1])
    opw_d = din("opw", [D, D])

    outT_d = nc.dram_tensor("outT", [D, T], f32, kind="ExternalOutput").ap()
    fsum_d = nc.dram_tensor("fsum", [E, 1], f32, kind="ExternalOutput").ap()
    Psum_d = nc.dram_tensor("Psum", [E, 1], f32, kind="ExternalOutput").ap()
    pre_d = nc.dram_tensor("pre_bounce", [D, T], f32r).ap()  # internal

    hT_r = hT_d.rearrange("(c p) t -> p c t", p=128)
    outT_r = outT_d.rearrange("(c p) t -> p c t", p=128)
    pre_r = pre_d.rearrange("(c p) t -> p c t", p=128)

    with tile.TileContext(nc) as tc:
        _build(nc, tc, hT_r, rwf_d, rb_d, ldf_d, lu_d, ew1_d, evth_d, ew2s_d,
               sw1f_d, svth_d, sw2s_d, sgwf_d, opw_d, outT_r, fsum_d, Psum_d,
               pre_r)
    nc.compile()
    return nc


def _build(nc, tc, hT_r, rwf_d, rb_d, ldf_d, lu_d, ew1_d, evth_d, ew2s_d,
           sw1f_d, svth_d, sw2s_d, sgwf_d, opw_d, outT_r, fsum_d, Psum_d,
           pre_r):
    from contextlib import ExitStack
    ctx = ExitStack()
    with ctx:
        # ---- long-lived pools -------------------------------------------
        const_p = ctx.enter_context(tc.tile_pool(name="const", bufs=1))
        hTn_p = ctx.enter_context(tc.tile_pool(name="hTn", bufs=1))
        small_p = ctx.enter_context(tc.tile_pool(name="small", bufs=1))
        racc_p = ctx.enter_context(tc.tile_pool(name="racc", bufs=1))

        ident = const_p.tile([128, 128], f32)
        masks.make_identity(nc, ident[:])
        ones_c = const_p.tile([128, 1], f32)
        nc.gpsimd.memset(ones_c[:], 1.0)
        eps_t = const_p.tile([1, 1], f32)
        nc.gpsimd.memset(eps_t[:], 1e-6)

        hTn = hTn_p.tile([128, DC, T], f32r)          # 64KB/p resident
        scoresT = small_p.tile([E, T], f32)
        comb = small_p.tile([E, T], f32)
        s_rep = small_p.tile([128, T], f32)
        sg_rep = small_p.tile([128, T], f32)
        latentT = small_p.tile([128, LC, T], f32r)
        racc_r = racc_p.tile([128, LC, T], f32r)

        # ---- phase 1: sum of squares ------------------------------------
        with tc.tile_pool(name="p1", bufs=3) as p1, \
             tc.tile_pool(name="p1ps", bufs=1, space="PSUM") as p1ps:
            ps_ss = p1ps.tile([1, T], f32)
            for c in range(DC):
                htc = p1.tile([128, T], f32, tag="htc")
                nc.sync.dma_start(htc[:], hT_r[:, c, :])
                sq = p1.tile([128, T], f32, tag="sq")
                nc.scalar.square(sq[:], htc[:])
                for nh in range(NH):
                    nc.tensor.matmul(ps_ss[:, nh * HALF:(nh + 1) * HALF],
                                     ones_c[:], sq[:, nh * HALF:(nh + 1) * HALF],
                                     start=(c == 0), stop=(c == DC - 1))
            # s = 1/sqrt(ms/D + eps)
            s_sqrt = p1.tile([1, T], f32, tag="s1")
            nc.scalar.activation(s_sqrt[:], ps_ss[:], AF.Sqrt,
                                 bias=eps_t[:], scale=1.0 / D)
            s_inv = p1.tile([1, T], f32, tag="s2")
            nc.vector.reciprocal(s_inv[:], s_sqrt[:])
            nc.gpsimd.partition_broadcast(s_rep[:], s_inv[:])

        # ---- phase 2: hTn = hT * s (f32r) --------------------------------
        with tc.tile_pool(name="p2", bufs=3) as p2:
            for c in range(DC):
                htc = p2.tile([128, T], f32, tag="htc2")
                nc.sync.dma_start(htc[:], hT_r[:, c, :])
                nc.vector.tensor_tensor(hTn[:, c, :], htc[:], s_rep[:],
                                        op=OP.mult)

        # ---- phase 3: router + shared gate ------------------------------
        with tc.tile_pool(name="p3", bufs=1) as p3, \
             tc.tile_pool(name="p3ps", bufs=1, space="PSUM") as p3ps:
            rwf_t = p3.tile([128, DC, E], f32r)
            nc.sync.dma_start(rwf_t[:], rwf_d.rearrange("(c p) e -> p c e", p=128))
            sgwf_t = p3.tile([128, DC, 1], f32r)
            nc.sync.dma_start(sgwf_t[:], sgwf_d.rearrange("(c p) e -> p c e", p=128))
            rb_t = p3.tile([E, 1], f32)
            nc.sync.dma_start(rb_t[:], rb_d[:])

            ps_r = p3ps.tile([E, T], f32)
            ps_g = p3ps.tile([1, T], f32)
            for nh in range(NH):
                sl = slice(nh * HALF, (nh + 1) * HALF)
                for c in range(DC):
                    nc.tensor.matmul(ps_r[:, sl], rwf_t[:, c, :],
                                     hTn[:, c, sl],
                                     start=(c == 0), stop=(c == DC - 1))
                for c in range(DC):
                    nc.tensor.matmul(ps_g[:, sl], sgwf_t[:, c, :],
                                     hTn[:, c, sl],
                                     start=(c == 0), stop=(c == DC - 1))
            # scores = sigmoid(logits + bias); P_partial = rowsum
            P_part = p3.tile([E, 1], f32)
            nc.scalar.activation(scoresT[:], ps_r[:], AF.Sigmoid,
                                 bias=rb_t[:], accum_out=P_part[:])
            nc.sync.dma_start(Psum_d[:], P_part[:])
            sgate = p3.tile([1, T], f32)
            nc.scalar.activation(sgate[:], ps_g[:], AF.Sigmoid, bias=0.0)
            nc.gpsimd.partition_broadcast(sg_rep[:], sgate[:])

        # ---- phase 4: top-k --------------------------------------------
        with tc.tile_pool(name="p4", bufs=2) as p4, \
             tc.tile_pool(name="p4ps", bufs=2, space="PSUM") as p4ps:
            for ti in range(T // 128):
                tsl = slice(ti * 128, (ti + 1) * 128)
                ps_t = p4ps.tile([128, E], f32, tag="pst")
                nc.tensor.transpose(ps_t[:], scoresT[:, tsl], ident[:E, :E])
                sc_t = p4.tile([128, E], f32, tag="sct")
                nc.scalar.copy(sc_t[:], ps_t[:])
                work = p4.tile([128, E], f32, tag="work")
                nc.vector.tensor_copy(work[:], sc_t[:])
                m = p4.tile([128, 1], f32, tag="m")
                for _ in range(TOPK - 1):
                    nc.vector.tensor_reduce(m[:], work[:], axis=AX.X, op=OP.max)
                    nc.vector.scalar_tensor_tensor(work[:], work[:], m[:],
                                                   work[:], op0=OP.is_lt,
                                                   op1=OP.mult)
                nc.vector.tensor_reduce(m[:], work[:], axis=AX.X, op=OP.max)
                gated = p4.tile([128, E], f32, tag="gated")
                nc.vector.scalar_tensor_tensor(gated[:], sc_t[:], m[:], sc_t[:],
                                               op0=OP.is_ge, op1=OP.mult)
                tsum = p4.tile([128, 1], f32, tag="tsum")
                nc.vector.tensor_reduce(tsum[:], gated[:], axis=AX.X, op=OP.add)
                denom = p4.tile([128, 1], f32, tag="denom")
                nc.scalar.activation(denom[:], tsum[:], AF.Copy, bias=1e-8)
                inv = p4.tile([128, 1], f32, tag="inv")
                nc.vector.reciprocal(inv[:], denom[:])
                ct = p4.tile([128, E], f32, tag="ct")
                nc.vector.tensor_scalar(out=ct[:], in0=gated[:], scalar1=inv[:],
                                        scalar2=None, op0=OP.mult)
                ps_c = p4ps.tile([E, 128], f32, tag="psc")
                nc.tensor.transpose(ps_c[:], ct[:], ident[:])
                nc.scalar.copy(comb[:, tsl], ps_c[:])
            # f_partial = rowsum(comb > 0)
            mask01 = p4.tile([E, T], f32, tag="mask")
            nc.vector.tensor_scalar(out=mask01[:], in0=comb[:], scalar1=0.0,
                                    scalar2=None, op0=OP.is_gt)
            f_part = p4.tile([E, 1], f32, tag="fp")
            nc.vector.tensor_reduce(f_part[:], mask01[:], axis=AX.X, op=OP.add)
            nc.sync.dma_start(fsum_d[:], f_part[:])

        # ---- phase 5: latent down --------------------------------------
        with tc.tile_pool(name="p5", bufs=1) as p5, \
             tc.tile_pool(name="p5ps", bufs=2, space="PSUM") as p5ps:
            ldf_t = p5.tile([128, DC, L], f32r)
            nc.sync.dma_start(ldf_t[:], ldf_d.rearrange("(c p) l -> p c l", p=128))
            for lc in range(LC):
                for nh in range(NH):
                    sl = slice(nh * HALF, (nh + 1) * HALF)
                    ps_l = p5ps.tile([128, HALF], f32, tag="psl")
                    for c in range(DC):
                        nc.tensor.matmul(ps_l[:], ldf_t[:, c, lc * 128:(lc + 1) * 128],
                                         hTn[:, c, sl],
                                         start=(c == 0), stop=(c == DC - 1))
                    nc.scalar.copy(latentT[:, lc, sl], ps_l[:])

        # ---- phase 6: experts (dense, comb-weighted) --------------------
        with tc.tile_pool(name="p6w", bufs=2) as p6w, \
             tc.tile_pool(name="p6", bufs=3) as p6, \
             tc.tile_pool(name="p6cw", bufs=2) as p6cw, \
             tc.tile_pool(name="p6r", bufs=1) as p6r, \
             tc.tile_pool(name="p6ps", bufs=2, space="PSUM") as p6ps, \
             tc.tile_pool(name="p6pso", bufs=2, space="PSUM") as p6pso:
            racc = p6r.tile([128, LC, T], f32)
            evth_t = const_p.tile([128, E, EHC], f32)
            nc.sync.dma_start(evth_t[:],
                              evth_d.rearrange("e (hc p) -> p e hc", p=128))
            for e in range(E):
                w1t = p6w.tile([128, LC, 2 * EH], f32r, tag="w1")
                nc.sync.dma_start(w1t[:],
                                  ew1_d[e].rearrange("(c p) m -> p c m", p=128))
                w2t = p6w.tile([128, EHC, L], f32r, tag="w2")
                nc.sync.dma_start(w2t[:],
                                  ew2s_d[e].rearrange("(c p) m -> p c m", p=128))
                cw_stage = p6cw.tile([1, T], f32, tag="cws")
                nc.sync.dma_start(cw_stage[:], comb[e:e + 1, :])
                cw_rep = p6cw.tile([128, T], f32, tag="cw")
                nc.gpsimd.partition_broadcast(cw_rep[:], cw_stage[:])
                for nh in range(NH):
                    sl = slice(nh * HALF, (nh + 1) * HALF)
                    ps_o = [p6pso.tile([128, HALF], f32, tag=f"pso{lc}",
                                       name=f"pso{lc}_{e}_{nh}")
                            for lc in range(LC)]
                    for hc in range(EHC):
                        ps_gt = p6ps.tile([128, HALF], f32, tag="psg")
                        ps_ln = p6ps.tile([128, HALF], f32, tag="psn")
                        for kc in range(LC):
                            nc.tensor.matmul(
                                ps_gt[:], w1t[:, kc, hc * 128:(hc + 1) * 128],
                                latentT[:, kc, sl],
                                start=(kc == 0), stop=(kc == LC - 1))
                        for kc in range(LC):
                            nc.tensor.matmul(
                                ps_ln[:],
                                w1t[:, kc, EH + hc * 128:EH + (hc + 1) * 128],
                                latentT[:, kc, sl],
                                start=(kc == 0), stop=(kc == LC - 1))
                        gsb = p6.tile([128, HALF], f32, tag="gsb")
                        nc.scalar.copy(gsb[:], ps_gt[:])
                        act = p6.tile([128, HALF], f32r, tag="act")
                        nc.vector.scalar_tensor_tensor(
                            act[:], gsb[:], evth_t[:, e, hc:hc + 1], ps_ln[:],
                            op0=OP.is_ge, op1=OP.mult)
                        for lc in range(LC):
                            nc.tensor.matmul(
                                ps_o[lc][:], w2t[:, hc, lc * 128:(lc + 1) * 128],
                                act[:],
                                start=(hc == 0), stop=(hc == EHC - 1))
                    for lc in range(LC):
                        if e == 0:
                            nc.vector.tensor_tensor(racc[:, lc, sl], ps_o[lc][:],
                                                    cw_rep[:, sl], op=OP.mult)
                        else:
                            t1 = p6.tile([128, HALF], f32, tag="t1")
                            nc.vector.tensor_tensor(t1[:], ps_o[lc][:],
                                                    cw_rep[:, sl], op=OP.mult)
                            nc.vector.tensor_tensor(racc[:, lc, sl],
                                                    racc[:, lc, sl], t1[:],
                                                    op=OP.add)
            for lc in range(LC):
                nc.scalar.copy(racc_r[:, lc, :], racc[:, lc, :])

        # ---- phase 7: shared expert + latent_up -> pre (DRAM bounce) ----
        with tc.tile_pool(name="p7w", bufs=2) as p7w, \
             tc.tile_pool(name="p7", bufs=3) as p7, \
             tc.tile_pool(name="p7s", bufs=1) as p7s, \
             tc.tile_pool(name="p7ps", bufs=2, space="PSUM") as p7ps, \
             tc.tile_pool(name="p7pso", bufs=2, space="PSUM") as p7pso:
            svth_t = const_p.tile([128, SHC], f32)
            nc.sync.dma_start(svth_t[:],
                              svth_d.rearrange("(hc p) -> p hc", p=128))
            lu_r = lu_d.rearrange("(c p) m -> p c m", p=128)
            sw1_r = sw1f_d.rearrange("(c p) m -> p c m", p=128)
            sw2_r = sw2s_d.rearrange("(c p) m -> p c m", p=128)
            for nh in range(NH):
                sl = slice(nh * HALF, (nh + 1) * HALF)
                sact2 = p7s.tile([128, SHC, HALF], f32r, tag="sact2")
                for hc in range(SHC):
                    ps_gt = p7ps.tile([128, HALF], f32, tag="psg7")
                    ps_ln = p7ps.tile([128, HALF], f32, tag="psn7")
                    w1g = p7w.tile([128, DC, 128], f32r, tag="w1g")
                    nc.sync.dma_start(w1g[:], sw1_r[:, :, hc * 128:(hc + 1) * 128])
                    w1l = p7w.tile([128, DC, 128], f32r, tag="w1l")
                    nc.sync.dma_start(
                        w1l[:], sw1_r[:, :, SH + hc * 128:SH + (hc + 1) * 128])
                    for kc in range(DC):
                        nc.tensor.matmul(ps_gt[:], w1g[:, kc, :], hTn[:, kc, sl],
                                         start=(kc == 0), stop=(kc == DC - 1))
                    for kc in range(DC):
                        nc.tensor.matmul(ps_ln[:], w1l[:, kc, :], hTn[:, kc, sl],
                                         start=(kc == 0), stop=(kc == DC - 1))
                    gsb = p7.tile([128, HALF], f32, tag="gsb7")
                    nc.scalar.copy(gsb[:], ps_gt[:])
                    sact = p7.tile([128, HALF], f32, tag="sact")
                    nc.vector.scalar_tensor_tensor(
                        sact[:], gsb[:], svth_t[:, hc:hc + 1], ps_ln[:],
                        op0=OP.is_ge, op1=OP.mult)
                    nc.vector.tensor_tensor(sact2[:, hc, :], sact[:],
                                            sg_rep[:, sl], op=OP.mult)
                for dc in range(DC):
                    ps_s = p7pso.tile([128, HALF], f32, tag="pss")
                    w2c = p7w.tile([128, SHC, 128], f32r, tag="w2c")
                    nc.sync.dma_start(w2c[:], sw2_r[:, :, dc * 128:(dc + 1) * 128])
                    lut = p7w.tile([128, LC, 128], f32r, tag="lut")
                    nc.sync.dma_start(lut[:], lu_r[:, :, dc * 128:(dc + 1) * 128])
                    for hc in range(SHC):
                        nc.tensor.matmul(ps_s[:], w2c[:, hc, :], sact2[:, hc, :],
                                         start=(hc == 0), stop=False)
                    for kc in range(LC):
                        nc.tensor.matmul(ps_s[:], lut[:, kc, :],
                                         racc_r[:, kc, sl],
                                         start=False, stop=(kc == LC - 1))
                    prc = p7.tile([128, HALF], f32r, tag="prc")
                    nc.scalar.copy(prc[:], ps_s[:])
                    nc.sync.dma_start(pre_r[:, dc, sl], prc[:])

        # ---- phase 8: out_proj + residual -------------------------------
        with tc.tile_pool(name="p8w", bufs=3) as p8w, \
             tc.tile_pool(name="p8pre", bufs=1) as p8pre, \
             tc.tile_pool(name="p8", bufs=4) as p8, \
             tc.tile_pool(name="p8ps", bufs=1, space="PSUM") as p8ps:
            opw_r = opw_d.rearrange("(c p) m -> p c m", p=128)
            for ts in range(NH):
                sl = slice(ts * HALF, (ts + 1) * HALF)
                pre_in = p8pre.tile([128, DC, HALF], f32r, tag="prein")
                nc.sync.dma_start(pre_in[:], pre_r[:, :, sl])
                for dp in range(2):           # dc groups of 8
                    ps_f = [p8ps.tile([128, HALF], f32, tag=f"psf{j}",
                                      name=f"psf{j}_{ts}_{dp}")
                            for j in range(8)]
                    for kc in range(DC):
                        opwt = p8w.tile([128, 1024], f32r, tag="opw")
                        nc.sync.dma_start(
                            opwt[:], opw_r[:, kc, dp * 1024:(dp + 1) * 1024])
                        for j in range(8):
                            nc.tensor.matmul(ps_f[j][:],
                                             opwt[:, j * 128:(j + 1) * 128],
                                             pre_in[:, kc, :],
                                             start=(kc == 0), stop=(kc == DC - 1))
                    for j in range(8):
                        dc = dp * 8 + j
                        hres = p8.tile([128, HALF], f32, tag="hres")
                        nc.sync.dma_start(hres[:], hT_r[:, dc, sl])
                        outf = p8.tile([128, HALF], f32, tag="outf")
                        nc.vector.tensor_tensor(outf[:], ps_f[j][:], hres[:],
                                                op=OP.add)
                        nc.sync.dma_start(outT_r[:, dc, sl], outf[:])


_NC = None


def _get_nc():
    global _NC
    if _NC is None:
        _NC = build_nc()
    return _NC


def _make_in_maps(inputs):
    h = np.asarray(inputs["h"], np.float32)
    norm_w = np.asarray(inputs["norm_w"], np.float32)
    nw = norm_w[:, None]
    shared_in = {
        "rwf": np.ascontiguousarray(nw * np.asarray(inputs["router_w"], np.float32)),
        "rb": np.ascontiguousarray(
            np.asarray(inputs["router_bias"], np.float32)[:, None]),
        "ldf": np.ascontiguousarray(
            nw * np.asarray(inputs["latent_down_w"], np.float32)),
        "lu": np.ascontiguousarray(np.asarray(inputs["latent_up_w"], np.float32)),
        "ew1": np.ascontiguousarray(np.asarray(inputs["expert_w1"], np.float32)),
        "evth": np.ascontiguousarray(np.asarray(inputs["expert_vth"], np.float32)),
        "ew2s": np.ascontiguousarray(
            np.asarray(inputs["expert_vth"], np.float32)[:, :, None]
            * np.asarray(inputs["expert_w2"], np.float32)),
        "sw1f": np.ascontiguousarray(
            nw * np.asarray(inputs["shared_w1"], np.float32)),
        "svth": np.ascontiguousarray(np.asarray(inputs["shared_vth"], np.float32)),
        "sw2s": np.ascontiguousarray(
            np.asarray(inputs["shared_vth"], np.float32)[:, None]
            * np.asarray(inputs["shared_w2"], np.float32)),
        "sgwf": np.ascontiguousarray(
            nw * np.asarray(inputs["shared_gate_w"], np.float32)),
        "opw": np.ascontiguousarray(np.asarray(inputs["out_proj_w"], np.float32)),
    }
    hf = h.reshape(N, D)
    in_maps = []
    for c in range(NCORES):
        hT = np.ascontiguousarray(hf[c * T:(c + 1) * T].T)
        in_maps.append({"hT": hT, **shared_in})
    return in_maps


def kernel(h, norm_w, latent_down_w, latent_up_w, router_w, router_bias,
           expert_w1, expert_vth, expert_w2,
           shared_w1, shared_vth, shared_w2, shared_gate_w, out_proj_w):
    in_maps = _make_in_maps(dict(
        h=h, norm_w=norm_w, latent_down_w=latent_down_w,
        latent_up_w=latent_up_w, router_w=router_w, router_bias=router_bias,
        expert_w1=expert_w1, expert_vth=expert_vth, expert_w2=expert_w2,
        shared_w1=shared_w1, shared_vth=shared_vth, shared_w2=shared_w2,
        shared_gate_w=shared_gate_w, out_proj_w=out_proj_w))

    nc = _get_nc()
    res = run_bass_kernel_spmd(nc, in_maps, list(range(NCORES)))

    out = np.empty((N, D), np.float32)
    ftot = np.zeros(E, np.float64)
    Ptot = np.zeros(E, np.float64)
    for c in range(NCORES):
        r = res.results[c]
        out[c * T:(c + 1) * T] = r["outT"].T
        ftot += r["fsum"][:, 0]
        Ptot += r["Psum"][:, 0]
    f = (ftot / N).astype(np.float32)
    P = (Ptot / N).astype(np.float32)
    lb = np.float32(E) * np.float32(np.sum(f * P)) * np.float32(1e-4)
    return out.reshape(S, B, D), np.float32(lb)
